# revision 1
# baseline (speedup 1.0000x reference)
"""GNN mean-aggregator (h = xW^T + b; out[i] = mean_{(i,j) in E} h[j]) on 8 trn2 cores.

Strategy (graph/data parallel over destination nodes):
  - Each core owns a contiguous range of 6250 destination nodes.
  - Host sorts edges by destination, groups them into 128-destination blocks,
    splits each block's edges by source-node half (int16 gather index limit),
    and pads each (block, half) group to whole 128-edge chunks, uniformly
    across cores (SPMD: one program, per-core data).
  - Device: dma_gather fetches fp16 x rows per edge (edge-major chunks),
    a one-hot matrix built with a single broadcast is_equal per gather maps
    edges to their local destination, and TensorE matmuls accumulate
    sum_{e} x[col_e] per destination block in PSUM (feature-major).
    A second small matmul applies W^T, then the result is scaled by 1/deg
    (and bias, masked for deg=0) and written out.
"""
import sys

sys.path.insert(0, "/opt/trn_rl_repo")

from contextlib import ExitStack

import numpy as np

from concourse import bass, bacc, mybir, tile
from concourse.bass_utils import run_bass_kernel_spmd

N_NODES = 50000
N_EDGES = 800000
D_IN = 128
D_OUT = 64
N_CORES = 8
NPC = N_NODES // N_CORES      # 6250 destination nodes per core
P = 128
NBLK = (NPC + P - 1) // P     # 49 blocks of 128 destinations
NPAD = NBLK * P               # 6272 padded destinations
HALF = 32768                  # int16 gather-index boundary
SB = 4                        # blocks per superblock (gather granularity)
NSB = (NBLK + SB - 1) // SB   # 13 superblocks

_prog_cache = {}
last_results = None  # test harness introspection


def _build_program(CA, CB):
    """CA/CB: per-block chunk counts (uniform across cores)."""
    CA = list(CA)
    CB = list(CB)
    CAtot = sum(CA)
    CBtot = sum(CB)

    nc = bacc.Bacc("TRN2", target_bir_lowering=False, debug=False,
                   num_swdge_queues=4, dynamic_dma_scratch_size=16384)
    f16 = mybir.dt.float16
    f32 = mybir.dt.float32
    i16 = mybir.dt.int16

    xlo = nc.declare_dram_parameter("xlo", [HALF, D_IN], f16, isOutput=False)
    xhi = nc.declare_dram_parameter("xhi", [N_NODES - HALF, D_IN], f16, isOutput=False)
    idxA = nc.declare_dram_parameter("idxA", [P, CAtot * 8], i16, isOutput=False)
    idxB = nc.declare_dram_parameter("idxB", [P, max(CBtot, 1) * 8], i16, isOutput=False)
    dlocA = nc.declare_dram_parameter("dlocA", [P, CAtot], f16, isOutput=False)
    dlocB = nc.declare_dram_parameter("dlocB", [P, max(CBtot, 1)], f16, isOutput=False)
    iota = nc.declare_dram_parameter("iota", [P, P], f16, isOutput=False)
    wt = nc.declare_dram_parameter("wt", [D_IN, D_OUT], f16, isOutput=False)
    scale = nc.declare_dram_parameter("scale", [D_OUT, NPAD], f32, isOutput=False)
    biasr = nc.declare_dram_parameter("biasr", [D_OUT, NPAD], f32, isOutput=False)
    outT = nc.declare_dram_parameter("outT", [D_OUT, NPAD], f32, isOutput=True)

    def bcast_mid(ap, reps):
        # [P, C] -> [P, C, reps] via zero-stride inner dim
        return bass.AP(tensor=ap.tensor, offset=ap.offset,
                       ap=[ap.ap[0], ap.ap[1], [0, reps]])

    def rep_mid(ap, reps):
        # [P, n] -> [P, reps, n] via zero-stride middle dim
        return bass.AP(tensor=ap.tensor, offset=ap.offset,
                       ap=[ap.ap[0], [0, reps], ap.ap[1]])

    with tile.TileContext(nc) as tc, ExitStack() as ctx:
        consts = ctx.enter_context(tc.tile_pool(name="consts", bufs=1))
        gxpA = ctx.enter_context(tc.tile_pool(name="gxA", bufs=3))
        gxpB = ctx.enter_context(tc.tile_pool(name="gxB", bufs=3))
        ohpA = ctx.enter_context(tc.tile_pool(name="ohA", bufs=3))
        ohpB = ctx.enter_context(tc.tile_pool(name="ohB", bufs=3))
        aggsb = ctx.enter_context(tc.tile_pool(name="aggsb", bufs=3))
        outsb = ctx.enter_context(tc.tile_pool(name="outsb", bufs=3))
        aggps = ctx.enter_context(tc.tile_pool(name="aggps", bufs=3, space="PSUM"))
        projps = ctx.enter_context(tc.tile_pool(name="projps", bufs=2, space="PSUM"))

        s_iota = consts.tile([P, P], f16)
        s_wt = consts.tile([D_IN, D_OUT], f16)
        s_idxA = consts.tile([P, CAtot * 8], i16)
        s_idxB = consts.tile([P, max(CBtot, 1) * 8], i16)
        s_dlocA = consts.tile([P, CAtot], f16)
        s_dlocB = consts.tile([P, max(CBtot, 1)], f16)
        s_scale = consts.tile([D_OUT, NPAD], f32)
        s_bias = consts.tile([D_OUT, NPAD], f32)
        nc.sync.dma_start(out=s_iota[:], in_=iota[:])
        nc.sync.dma_start(out=s_wt[:], in_=wt[:])
        nc.sync.dma_start(out=s_idxA[:], in_=idxA[:])
        nc.sync.dma_start(out=s_idxB[:], in_=idxB[:])
        nc.sync.dma_start(out=s_dlocA[:], in_=dlocA[:])
        nc.sync.dma_start(out=s_dlocB[:], in_=dlocB[:])
        nc.sync.dma_start(out=s_scale[:], in_=scale[:])
        nc.sync.dma_start(out=s_bias[:], in_=biasr[:])

        offA = 0
        offB = 0
        qctr = [0]
        for sb in range(NSB):
            blocks = list(range(sb * SB, min(sb * SB + SB, NBLK)))
            nb = len(blocks)
            ca = [CA[b] for b in blocks]
            cb = [CB[b] for b in blocks]
            casb = sum(ca)
            cbsb = sum(cb)

            gxA = gxpA.tile([P, casb, D_IN], f16, tag="gxA")
            nsegA = -(-casb // 16)
            s0 = 0
            for g in range(nsegA):
                seg = (casb - s0) // (nsegA - g)
                nc.gpsimd.dma_gather(
                    gxA[:, s0 : s0 + seg, :], xlo[:],
                    s_idxA[:, (offA + s0) * 8 : (offA + s0 + seg) * 8],
                    seg * P, seg * P, D_IN, single_packet=False,
                    queue_num=qctr[0] % 4,
                )
                qctr[0] += 1
                s0 += seg
            ohA = ohpA.tile([P, casb, P], f16, tag="ohA")
            nc.vector.tensor_tensor(
                out=ohA[:],
                in0=bcast_mid(s_dlocA[:, offA : offA + casb], P),
                in1=rep_mid(s_iota[:], casb),
                op=mybir.AluOpType.is_equal,
            )
            if cbsb > 0:
                gxB = gxpB.tile([P, cbsb, D_IN], f16, tag="gxB")
                nsegB = -(-cbsb // 16)
                s0 = 0
                for g in range(nsegB):
                    seg = (cbsb - s0) // (nsegB - g)
                    nc.gpsimd.dma_gather(
                        gxB[:, s0 : s0 + seg, :], xhi[:],
                        s_idxB[:, (offB + s0) * 8 : (offB + s0 + seg) * 8],
                        seg * P, seg * P, D_IN, single_packet=False,
                        queue_num=qctr[0] % 4,
                    )
                    qctr[0] += 1
                    s0 += seg
                ohB = ohpB.tile([P, cbsb, P], f16, tag="ohB")
                nc.vector.tensor_tensor(
                    out=ohB[:],
                    in0=bcast_mid(s_dlocB[:, offB : offB + cbsb], P),
                    in1=rep_mid(s_iota[:], cbsb),
                    op=mybir.AluOpType.is_equal,
                )

            agg_ps = aggps.tile([P, nb * P], f32, space="PSUM", tag="aggps")
            a0 = 0
            b0 = 0
            for bl in range(nb):
                nchunks = ca[bl] + cb[bl]
                j = 0
                for c in range(ca[bl]):
                    nc.tensor.matmul(
                        agg_ps[:, bl * P : (bl + 1) * P],
                        lhsT=gxA[:, a0 + c, :],
                        rhs=ohA[:, a0 + c, :],
                        start=(j == 0),
                        stop=(j == nchunks - 1),
                    )
                    j += 1
                for c in range(cb[bl]):
                    nc.tensor.matmul(
                        agg_ps[:, bl * P : (bl + 1) * P],
                        lhsT=gxB[:, b0 + c, :],
                        rhs=ohB[:, b0 + c, :],
                        start=(j == 0),
                        stop=(j == nchunks - 1),
                    )
                    j += 1
                a0 += ca[bl]
                b0 += cb[bl]

            agg_s = aggsb.tile([P, nb * P], f16, tag="aggsb")
            nc.scalar.copy(out=agg_s[:], in_=agg_ps[:])

            proj_ps = projps.tile([D_OUT, nb * P], f32, space="PSUM", tag="projps")
            nc.tensor.matmul(proj_ps[:], lhsT=s_wt[:], rhs=agg_s[:],
                             start=True, stop=True)

            out_s = outsb.tile([D_OUT, nb * P], f32, tag="outsb")
            colsl = slice(sb * SB * P, sb * SB * P + nb * P)
            nc.vector.tensor_tensor(out=out_s[:], in0=proj_ps[:],
                                    in1=s_scale[:, colsl], op=mybir.AluOpType.mult)
            nc.vector.tensor_tensor(out=out_s[:], in0=out_s[:],
                                    in1=s_bias[:, colsl], op=mybir.AluOpType.add)
            nc.sync.dma_start(out=outT[:, colsl], in_=out_s[:])

            offA += casb
            offB += cbsb

    nc.compile()
    return nc


def _wrap_idx(idx_list):
    """[n] int16 -> [128, n//16] wrapped + replicated layout."""
    n = idx_list.shape[0]
    w16 = idx_list.reshape(n // 16, 16).T  # [16, n/16]
    return np.tile(w16, (8, 1)).astype(np.int16)


def kernel(x, W, b, row, col):
    global last_results
    x = np.asarray(x, dtype=np.float32)
    W = np.asarray(W, dtype=np.float32)
    b = np.asarray(b, dtype=np.float32)
    row = np.asarray(row).astype(np.int64)
    col = np.asarray(col).astype(np.int64)

    deg = np.bincount(row, minlength=N_NODES)
    recip = np.where(deg > 0, 1.0 / np.maximum(deg, 1), 0.0).astype(np.float32)
    mask = (deg > 0).astype(np.float32)

    # sort edges by (core, block, half)
    core = row // NPC
    local = row - core * NPC
    blk = local // P
    dloc = (local - blk * P).astype(np.int16)
    half = (col >= HALF).astype(np.int64)
    key = (core * NBLK + blk) * 2 + half
    order = np.argsort(key, kind="stable")
    ks = key[order]
    cs = col[order]
    dl = dloc[order]

    counts = np.bincount(ks, minlength=N_CORES * NBLK * 2).reshape(N_CORES, NBLK, 2)
    chunks = -(-counts // P)  # ceil
    CA = np.maximum(chunks[:, :, 0].max(axis=0), 1)  # [NBLK]
    CB = chunks[:, :, 1].max(axis=0)                 # [NBLK]
    CAtot = int(CA.sum())
    CBtot = int(CB.sum())

    starts = np.zeros(N_CORES * NBLK * 2 + 1, np.int64)
    np.cumsum(counts.reshape(-1), out=starts[1:])

    # per-core padded streams
    idxA_dev = np.zeros((N_CORES, P, CAtot * 8), np.int16)
    idxB_dev = np.zeros((N_CORES, P, max(CBtot, 1) * 8), np.int16)
    dlocA_dev = np.zeros((N_CORES, P, CAtot), np.float16)
    dlocB_dev = np.zeros((N_CORES, P, max(CBtot, 1)), np.float16)
    scale_dev = np.zeros((N_CORES, D_OUT, NPAD), np.float32)
    bias_dev = np.zeros((N_CORES, D_OUT, NPAD), np.float32)

    for k in range(N_CORES):
        for h, (Cb, idx_dev, dloc_dev, base_sub) in enumerate(
            ((CA, idxA_dev, dlocA_dev, 0), (CB, idxB_dev, dlocB_dev, HALF))
        ):
            idx_stream = np.zeros(int(Cb.sum()) * P, np.int16)
            dl_stream = np.full(int(Cb.sum()) * P, -1.0, np.float16)
            off = 0
            for bidx in range(NBLK):
                g = (k * NBLK + bidx) * 2 + h
                s, e = starts[g], starts[g + 1]
                n = e - s
                idx_stream[off : off + n] = (cs[s:e] - base_sub).astype(np.int16)
                dl_stream[off : off + n] = dl[s:e].astype(np.float16)
                off += int(Cb[bidx]) * P
            if Cb.sum() == 0:
                continue
            # wrap per superblock call
            woff = 0
            soff = 0
            for sb in range(NSB):
                blocks = range(sb * SB, min(sb * SB + SB, NBLK))
                csb = int(sum(Cb[bb] for bb in blocks))
                if csb == 0:
                    continue
                n = csb * P
                idx_dev[k][:, woff * 8 : woff * 8 + n // 16] = _wrap_idx(
                    idx_stream[soff : soff + n]
                )
                woff += csb
                soff += n
            dloc_dev[k] = dl_stream.reshape(-1, P).T
        base = k * NPC
        scale_dev[k][:, :NPC] = recip[base : base + NPC][None, :]
        bias_dev[k][:, :NPC] = b[:, None] * mask[None, base : base + NPC]

    xlo = np.ascontiguousarray(x[:HALF]).astype(np.float16)
    xhi = np.ascontiguousarray(x[HALF:]).astype(np.float16)
    iota_t = np.tile(np.arange(P, dtype=np.float16), (P, 1))
    wt = np.ascontiguousarray(W.T).astype(np.float16)

    in_maps = []
    for k in range(N_CORES):
        in_maps.append(
            dict(
                xlo=xlo, xhi=xhi,
                idxA=idxA_dev[k], idxB=idxB_dev[k],
                dlocA=dlocA_dev[k], dlocB=dlocB_dev[k],
                iota=iota_t, wt=wt,
                scale=scale_dev[k], biasr=bias_dev[k],
            )
        )

    cache_key = (tuple(CA.tolist()), tuple(CB.tolist()))
    if cache_key not in _prog_cache:
        _prog_cache[cache_key] = _build_program(CA, CB)
    nc = _prog_cache[cache_key]

    res = run_bass_kernel_spmd(nc, in_maps, core_ids=list(range(N_CORES)))
    last_results = res

    out = np.empty((N_NODES, D_OUT), np.float32)
    for k in range(N_CORES):
        out[k * NPC : (k + 1) * NPC] = res.results[k]["outT"][:, :NPC].T
    return out

